# revision 8
# baseline (speedup 1.0000x reference)
"""Trainium2 Bass kernel for the DFBL (Gabor filterbank + Kaiser pooling + PCEN) model.

Contract: kernel(**inputs) takes the FULL unsharded inputs
(x [8,1,160000], six [64] param vectors) and returns the FULL output
[8, 64, 1000] float32. Internally shards batch across 8 NeuronCores.

Algorithm (per core, one batch element):
  1. Gabor conv as matmuls via the residue decomposition t = 128u + s:
     out[n, 128u+s] = sum_d Wsd[q,n].T @ x2[q, u+d], where
     x2[q, c] = xpad[128c + q] is a time-minor layout of x loaded once,
     and Wsd are fp8e4m3 weight tiles (pow2 per-channel-pair scaled; the
     scale never needs undoing because PCEN is per-channel scale
     invariant). All weights stay SBUF-resident (8.7 MB).
  2. |.|^2 on the scalar engine into per-group s-major buffers (bf16).
  3. Block-sum pooling: the Kaiser window is approximated piecewise-
     constant over blocks of 8 samples (KB[j] = block mean). Because
     t = 128u + s, a block of 8 consecutive samples = 8 consecutive
     residues s at fixed u, so one grouped tensor_reduce per 8-residue
     group produces S8[m], m = 16u + g, written m-contiguous. This
     shrinks the transpose+pool-matmul PE work ~8x vs pooling at full
     rate.
  4. pooledT[tp, chan] accumulates in persistent PSUM banks via banded
     KB tiles (22 distinct [128,128] bf16 tiles).
  5. PCEN: EMA as a bf16 decay-matrix matmul, then the elementwise pow
     chain on ACT/DVE (same structure as the reference scan unrolled).
"""

import math
import os

import ml_dtypes
import numpy as np

SR = 16000
NF = 64
GK = 401
PK = 401
PSTRIDE = 160
PCEN_S = 0.025
FMIN = 30.0
FMAX = SR / 2.0 * 0.5
B, T = 8, 160000
TP = 1000
U = 1250  # T / 128
X2C = 1254  # x2 columns: u+d+2 for u<1250, d in [-2,2]
N_CORES = 8

BLK = 8           # pooling block size (samples)
NMB = T // BLK    # 20000 pooled blocks per core
KBN = (PK + BLK - 1) // BLK  # 51 block-kernel taps
STRB = PSTRIDE // BLK        # 20 blocks per pooled-output stride
OFFB = (PK // 2) // BLK      # 25 blocks of left context
SEGS = [(0, 256), (256, 512), (512, 768), (768, 1024), (1024, 1250)]

BF16 = ml_dtypes.bfloat16
E4M3 = ml_dtypes.float8_e4m3

# exposed for test.py
LAST_RESULT = None
LAST_NC = None
LAST_IN_MAPS = None


# ----------------------------------------------------------------- host math

def _softplus(x):
    return np.logaddexp(0.0, x)


def _host_filters(p_center, p_bw):
    """Wcat [128, 401] f32: rows 0-63 real, 64-127 imag, scaled by sqrt(0.5)."""
    half = (GK - 1) // 2
    t = np.arange(-half, half + 1, dtype=np.float64) / SR
    fc = np.clip(np.exp(p_center.astype(np.float64)), FMIN, FMAX - 10.0)
    bw_pos = _softplus(p_bw.astype(np.float64)) * 1000.0
    max_bw = 2.0 * np.minimum(fc - FMIN, FMAX - fc)
    bw = np.minimum(bw_pos, np.maximum(max_bw, 50.0))
    f_low = np.maximum(fc - 0.5 * bw, FMIN)
    f_high = np.minimum(fc + 0.5 * bw, FMAX)
    sigma = 0.5 / np.maximum(f_high - f_low, 20.0)
    env = np.exp(-0.5 * (t[None, :] / sigma[:, None]) ** 2)
    phase = 2.0 * np.pi * fc[:, None] * t[None, :]
    real_k = env * np.cos(phase)
    imag_k = env * np.sin(phase)
    W = np.concatenate([real_k, imag_k], axis=0) * np.sqrt(0.5)
    return W.astype(np.float32)


def _host_kaiser(beta):
    b = np.clip(beta.astype(np.float64), 1.0, 20.0)
    n = np.arange(PK, dtype=np.float64)
    arg = b[:, None] * np.sqrt(1.0 - (2.0 * n[None, :] / (PK - 1.0) - 1.0) ** 2)
    kais = np.i0(arg) / (np.i0(b)[:, None] + 1e-8)
    return kais.astype(np.float64)


def _valid_d(s):
    lo = int(math.ceil((s - 327) / 128))
    hi = (s + 200) // 128
    return list(range(lo, hi + 1))


def _woff():
    off, acc = [], 0
    for s in range(128):
        off.append(acc)
        acc += len(_valid_d(s))
    return off, acc


def _build_weight_array(W):
    """W_all [128, ntiles*128] fp8e4m3, tiles ordered (s asc, d asc).

    Per-channel pow2 scale (shared between the re/im of each filter, so
    re^2+im^2 stays consistently scaled); PCEN is per-channel scale
    invariant so the scale is never undone."""
    wmax = np.abs(W).max(axis=1)
    wmax = np.maximum(wmax[:NF], wmax[NF:])
    wmax = np.maximum(wmax, 1e-30)
    alpha = 2.0 ** np.floor(np.log2(176.0 / wmax))
    alpha = np.concatenate([alpha, alpha]).astype(np.float32)
    Ws = W * alpha[:, None]
    tiles = []
    for s in range(128):
        for d in _valid_d(s):
            tile = np.zeros((128, 128), np.float32)
            q = np.arange(128)
            k = 128 * d + q + 200 - s
            msk = (k >= 0) & (k < GK)
            tile[msk, :] = Ws[:, k[msk]].T
            tiles.append(tile)
    return np.concatenate(tiles, axis=1).astype(E4M3)


def _build_kbb(kr):
    """Banded block-kernel tiles [128, 22*128] bf16.

    KB[j] = mean of kr over block j (zero padded). Tile t (offset
    o = 128*(t-1)): T[m, tp] = KB[o + m - 20*tp + 25]."""
    kr_pad = np.zeros(KBN * BLK)
    kr_pad[:PK] = kr
    KB = kr_pad.reshape(KBN, BLK).mean(axis=1)
    m = np.arange(128)
    tp = np.arange(128)
    tiles = []
    for t in range(22):
        o = 128 * (t - 1)
        idx = o + m[:, None] - 20 * tp[None, :] + OFFB
        tile = np.where((idx >= 0) & (idx < KBN), KB[np.clip(idx, 0, KBN - 1)], 0.0)
        tiles.append(tile)
    return np.concatenate(tiles, axis=1).astype(BF16)


def _build_L():
    k_idx = np.arange(1024)
    tp_idx = np.arange(TP)
    Lm = np.where(
        (k_idx[:, None] <= tp_idx[None, :]) & (k_idx[:, None] < TP),
        PCEN_S * (1.0 - PCEN_S) ** np.clip(tp_idx[None, :] - k_idx[:, None], 0, None),
        0.0,
    )
    Ld = np.zeros((128, 8 * TP), np.float32)
    for blk in range(8):
        Ld[:, blk * TP:(blk + 1) * TP] = Lm[blk * 128:(blk + 1) * 128, :]
    return Ld.astype(BF16)


def _chunk_blocks():
    """Global pooling schedule: list of (C, mc, [(b, kbb_tile_idx), ...])."""
    out = []
    for C in range(157):
        m0 = 128 * C
        mc = min(128, NMB - m0)
        tp_lo = max(0, -(-(m0 - OFFB) // STRB))
        tp_hi = min(TP - 1, (m0 + mc - 1 + OFFB) // STRB)
        pairs = []
        if tp_lo <= tp_hi:
            for b in range(tp_lo // 128, tp_hi // 128 + 1):
                t = C - 20 * b + 1
                pairs.append((b, t))
        out.append((C, mc, pairs))
    return out


# ------------------------------------------------------------- device kernel

def _build_program():
    import concourse.bacc as bacc
    import concourse.bass as bass
    import concourse.mybir as mybir
    import concourse.tile as tile
    from concourse._compat import axon_active

    f32 = mybir.dt.float32
    bf16 = mybir.dt.bfloat16
    fp8 = mybir.dt.float8e4
    AF = mybir.ActivationFunctionType
    ALU = mybir.AluOpType

    woff, n_wtiles = _woff()
    sched = _chunk_blocks()

    # first/last pooling contribution per PSUM BANK (start zeroes the whole
    # bank, so flags must be bank-granular like the baseline)
    blk_first, blk_last = {}, {}
    for C, mc, pairs in sched:
        for b, t in pairs:
            bank = b // 4
            if bank not in blk_first:
                blk_first[bank] = (C, b)
            blk_last[bank] = (C, b)

    nc = bacc.Bacc(
        "TRN2",
        target_bir_lowering=False,
        debug=not axon_active(),
        num_devices=N_CORES,
    )

    x2_d = nc.dram_tensor("x2", [128, X2C], bf16, kind="ExternalInput").ap()
    w_d = nc.dram_tensor("W", [128, n_wtiles * 128], fp8, kind="ExternalInput").ap()
    kbb_d = nc.dram_tensor("KBB", [128, 22 * 128], bf16, kind="ExternalInput").ap()
    idb_d = nc.dram_tensor("IDB", [128, 128], bf16, kind="ExternalInput").ap()
    idf_d = nc.dram_tensor("IDF", [128, 128], f32, kind="ExternalInput").ap()
    par_d = nc.dram_tensor("PAR", [64, 5], f32, kind="ExternalInput").ap()
    l_d = nc.dram_tensor("L", [128, 8 * TP], bf16, kind="ExternalInput").ap()
    y_d = nc.dram_tensor("Y", [64, TP], f32, kind="ExternalOutput").ap()
    dbg = bool(int(os.environ.get("DFBL_DEBUG", "0")))
    if dbg:
        sq0_d = nc.dram_tensor("DSQ0", [128, 2048], bf16, kind="ExternalOutput").ap()
        sct_d = nc.dram_tensor("DSCT", [128, 512], bf16, kind="ExternalOutput").ap()
        sg0_d = nc.dram_tensor("DSG0", [128, 4096], bf16, kind="ExternalOutput").ap()
        ps_d = nc.dram_tensor("DPS", [128, 512], f32, kind="ExternalOutput").ap()
        ema_d = nc.dram_tensor("DEMA", [64, TP], f32, kind="ExternalOutput").ap()

    with tile.TileContext(nc) as tc:
        with (
            tc.tile_pool(name="const", bufs=1) as const_pool,
            tc.tile_pool(name="sqb", bufs=2) as sq_pool,
            tc.tile_pool(name="sg", bufs=2) as sg_pool,
            tc.tile_pool(name="sct", bufs=3) as sct_pool,
            tc.tile_pool(name="misc", bufs=1) as misc_pool,
            tc.tile_pool(name="psA", bufs=4, space="PSUM") as psA,
            tc.tile_pool(name="psB", bufs=2, space="PSUM") as psB,
            tc.tile_pool(name="psC", bufs=1, space="PSUM") as psC,
        ):
            x2_sb = const_pool.tile([128, X2C], bf16, tag="x2")
            nc.sync.dma_start(x2_sb[:, 0:384], x2_d[:, 0:384])
            nc.sync.dma_start(x2_sb[:, 384:X2C], x2_d[:, 384:X2C])
            w_sb = const_pool.tile([128, n_wtiles * 128], fp8, tag="w")
            # W chunks ordered by s-groups so conv can start early
            wg_bounds = [woff[g] if g < 128 else n_wtiles for g in range(0, 129, 8)]
            for i in range(16):
                lo, hi = wg_bounds[i] * 128, wg_bounds[i + 1] * 128
                nc.sync.dma_start(w_sb[:, lo:hi], w_d[:, lo:hi])
            kbb_sb = const_pool.tile([128, 22 * 128], bf16, tag="kbb")
            nc.sync.dma_start(kbb_sb[:], kbb_d[:])
            idb_sb = const_pool.tile([128, 128], bf16, tag="idb")
            nc.sync.dma_start(idb_sb[:], idb_d[:])
            idf_sb = const_pool.tile([128, 128], f32, tag="idf")
            nc.sync.dma_start(idf_sb[:], idf_d[:])
            par_sb = const_pool.tile([64, 5], f32, tag="par")
            nc.sync.dma_start(par_sb[:], par_d[:])
            l_sb = const_pool.tile([128, 8 * TP], bf16, tag="L")
            nc.sync.dma_start(l_sb[:], l_d[:])

            pooled_ps = [
                psC.tile([128, 512], f32, tag=f"pool{i}", name=f"pool{i}")
                for i in range(2)
            ]

            gchunk = 0  # global chunk counter
            for (u0, u1) in SEGS:
                useg = u1 - u0
                sg = sg_pool.tile([128, 16 * 256], bf16, tag="sg", name="sg")
                sgv = sg[:].rearrange("p (u g) -> p u g", g=16)

                for g in range(16):  # 8-residue groups
                    sqb = sq_pool.tile([128, 8 * 256], bf16, tag="sqb", name="sqb")
                    for q in range(4):  # 2 s per PSUM bank
                        cps = psA.tile([128, 512], f32, tag="conv", name="cps")
                        for j in range(2):
                            s = 8 * g + 2 * q + j
                            ds = _valid_d(s)
                            for di, d in enumerate(ds):
                                nc.tensor.matmul(
                                    cps[:, j * useg : (j + 1) * useg],
                                    lhsT=w_sb[
                                        :,
                                        (woff[s] + di) * 128 : (woff[s] + di + 1) * 128,
                                    ],
                                    rhs=x2_sb[:, u0 + d + 2 : u0 + d + 2 + useg],
                                    start=(di == 0),
                                    stop=(di == len(ds) - 1),
                                    skip_group_check=True,
                                )
                        nc.scalar.activation(
                            sqb[:, 2 * q * useg : (2 * q + 2) * useg],
                            cps[:, 0 : 2 * useg],
                            AF.Square,
                        )
                    if dbg and u0 == 0 and g == 0:
                        nc.sync.dma_start(sq0_d[:], sqb[:, 0:2048])
                    sqv = sqb[:, 0 : 8 * useg].rearrange("p (s u) -> p u s", s=8)
                    with nc.allow_low_precision(reason="8-sample pooling blocksum"):
                        nc.vector.tensor_reduce(
                            sgv[:, 0:useg, g],
                            sqv,
                            op=ALU.add,
                            axis=mybir.AxisListType.X,
                        )

                if dbg and u0 == 0:
                    nc.sync.dma_start(sg0_d[:], sg[:])
                # pooling for this segment
                nchunk = (16 * useg) // 128
                mrem = 16 * useg - nchunk * 128
                for cb in range(0, nchunk, 4):
                    n4 = min(4, nchunk - cb)
                    tp_ps = psB.tile([128, 512], bf16, tag="tp", name="tpps")
                    for j in range(n4):
                        nc.tensor.transpose(
                            tp_ps[:, j * 128 : (j + 1) * 128],
                            sg[:, (cb + j) * 128 : (cb + j + 1) * 128],
                            idb_sb[:],
                        )
                    sct = sct_pool.tile([128, 512], bf16, tag="sct", name="sct")
                    nc.vector.tensor_copy(sct[:, 0 : n4 * 128], tp_ps[:, 0 : n4 * 128])
                    if dbg and u0 == 0 and cb == 0:
                        nc.sync.dma_start(sct_d[:], sct[:])
                    for j in range(n4):
                        C, mc, pairs = sched[gchunk + cb + j]
                        for b, t in pairs:
                            nc.tensor.matmul(
                                pooled_ps[b // 4][:, (b % 4) * 128 : (b % 4 + 1) * 128],
                                lhsT=kbb_sb[:, t * 128 : (t + 1) * 128],
                                rhs=sct[:, j * 128 : (j + 1) * 128],
                                start=(blk_first[b // 4] == (C, b)),
                                stop=(blk_last[b // 4] == (C, b)),
                                skip_group_check=True,
                            )
                if mrem:
                    # partial chunk (last segment): mrem m-columns
                    tp_ps = psB.tile([128, 512], bf16, tag="tp", name="tpps")
                    nc.tensor.transpose(
                        tp_ps[0:mrem, 0:128],
                        sg[:, nchunk * 128 : nchunk * 128 + mrem],
                        idb_sb[:],
                    )
                    sct = sct_pool.tile([128, 512], bf16, tag="sct", name="sct")
                    nc.vector.tensor_copy(sct[0:mrem, 0:128], tp_ps[0:mrem, 0:128])
                    C, mc, pairs = sched[gchunk + nchunk]
                    assert mc == mrem
                    for b, t in pairs:
                        nc.tensor.matmul(
                            pooled_ps[b // 4][:, (b % 4) * 128 : (b % 4 + 1) * 128],
                            lhsT=kbb_sb[0:mrem, t * 128 : (t + 1) * 128],
                            rhs=sct[0:mrem, 0:128],
                            start=(blk_first[b // 4] == (C, b)),
                            stop=(blk_last[b // 4] == (C, b)),
                            skip_group_check=True,
                        )
                    gchunk += 1
                gchunk += nchunk

            # ---- PCEN tail ----
            pc = []
            for i in range(2):
                t = misc_pool.tile([128, 512], f32, tag=f"pc{i}")
                nc.vector.tensor_copy(t[:], pooled_ps[i][:])
                pc.append(t)
            poolsumT = misc_pool.tile([128, 512], f32, tag="pst")
            for blk in range(8):
                src = pc[blk // 4]
                col = (blk % 4) * 128
                nc.vector.tensor_add(
                    poolsumT[:, blk * 64 : (blk + 1) * 64],
                    src[:, col : col + 64],
                    src[:, col + 64 : col + 128],
                )
            if dbg:
                nc.sync.dma_start(ps_d[:], poolsumT[:])
            poolsumTb = misc_pool.tile([128, 512], bf16, tag="pstb")
            nc.gpsimd.tensor_copy(poolsumTb[:], poolsumT[:])

            ema_ps = [
                psA.tile([64, 500], f32, tag="conv", name=f"ema{_i}") for _i in range(2)
            ]
            for blk in range(8):
                for half in range(2):
                    nc.tensor.matmul(
                        ema_ps[half][:],
                        lhsT=poolsumTb[:, blk * 64 : (blk + 1) * 64],
                        rhs=l_sb[:, blk * TP + half * 500 : blk * TP + (half + 1) * 500],
                        start=(blk == 0),
                        stop=(blk == 7),
                    )

            pnm_ps = [
                psB.tile([64, 512], f32, tag="tp", name=f"pnm{_i}") for _i in range(2)
            ]
            for blk in range(8):
                nc.tensor.transpose(
                    pnm_ps[blk // 4][:, (blk % 4) * 128 : (blk % 4 + 1) * 128],
                    poolsumT[:, blk * 64 : (blk + 1) * 64],
                    idf_sb[:],
                )

            if dbg:
                emad = misc_pool.tile([64, TP], f32, tag="emad")
                nc.scalar.copy(emad[:, 0:500], ema_ps[0][:])
                nc.scalar.copy(emad[:, 500:TP], ema_ps[1][:])
                nc.sync.dma_start(ema_d[:], emad[:])
            t0 = misc_pool.tile([64, TP], f32, tag="t0")
            rec = misc_pool.tile([64, TP], f32, tag="rec")
            pnm = misc_pool.tile([64, TP], f32, tag="pnm")
            nc.scalar.copy(pnm[:, 0:512], pnm_ps[0][:])
            nc.scalar.copy(pnm[:, 512:TP], pnm_ps[1][:, 0:488])
            t2 = misc_pool.tile([64, TP], f32, tag="t2")
            t3 = misc_pool.tile([64, TP], f32, tag="t3")
            t4 = misc_pool.tile([64, TP], f32, tag="t4")
            y_sb = misc_pool.tile([64, TP], f32, tag="y")
            for half in range(2):
                hs = slice(half * 500, (half + 1) * 500)
                nc.scalar.activation(
                    t0[:, hs], ema_ps[half][:], AF.Identity, bias=par_sb[:, 4:5]
                )
                nc.vector.reciprocal(rec[:, hs], t0[:, hs])
                nc.vector.tensor_mul(t2[:, hs], pnm[:, hs], rec[:, hs])
                nc.scalar.activation(
                    t3[:, hs], t2[:, hs], AF.Ln, bias=par_sb[:, 0:1], scale=1.0
                )
                nc.scalar.activation(
                    t4[:, hs], t3[:, hs], AF.Exp, bias=0.0, scale=par_sb[:, 1:2]
                )
                nc.vector.tensor_scalar(
                    y_sb[:, hs], t4[:, hs], par_sb[:, 2:3], par_sb[:, 3:4],
                    ALU.mult, ALU.subtract,
                )
                nc.sync.dma_start(y_d[:, hs], y_sb[:, hs])

    nc.compile()
    return nc


def _numpy_fallback(x, W, kais, pcen_g, pcen_o, pcen_e):
    """Correct-but-slow host path for non-uniform beta (never hit with the
    harness inputs, which use a uniform beta)."""
    out = np.zeros((B, NF, TP), np.float32)
    Wr, Wi = W[:NF] / np.sqrt(0.5), W[NF:] / np.sqrt(0.5)
    for b in range(B):
        xp = np.zeros(T + 2 * 200, np.float32)
        xp[200 : 200 + T] = x[b, 0]
        win = np.lib.stride_tricks.sliding_window_view(xp, GK)  # [T, GK]
        real = win @ Wr.T
        imag = win @ Wi.T
        scal = 0.5 * (real ** 2 + imag ** 2)  # [T, NF]
        sp = np.zeros((T + 2 * 200, NF), np.float32)
        sp[200 : 200 + T] = scal
        pooled = np.zeros((TP, NF), np.float32)
        for tp in range(TP):
            seg = sp[tp * PSTRIDE : tp * PSTRIDE + PK]
            pooled[tp] = np.einsum("kn,nk->n", seg, kais)
        g = np.clip(pcen_g, 0.5, 0.999)
        o = np.clip(pcen_o, 0.0, 10.0)
        e = np.clip(pcen_e, 0.1, 1.0)
        ema = np.zeros(NF, np.float32)
        for tp in range(TP):
            ema = (1.0 - PCEN_S) * ema + PCEN_S * pooled[tp]
            out[b, :, tp] = ((pooled[tp] / (ema + 1e-6) + o) ** e - o ** e) * g
    return out


def kernel(x, p_center, p_bw, beta, pcen_g, pcen_o, pcen_e):
    global LAST_RESULT
    x = np.asarray(x, np.float32)
    p_center = np.asarray(p_center, np.float32)
    p_bw = np.asarray(p_bw, np.float32)
    beta = np.asarray(beta, np.float32)
    pcen_g = np.asarray(pcen_g, np.float32)
    pcen_o = np.asarray(pcen_o, np.float32)
    pcen_e = np.asarray(pcen_e, np.float32)

    W = _host_filters(p_center, p_bw)
    kais = _host_kaiser(beta)
    if not np.all(kais == kais[0:1]):
        return _numpy_fallback(x, W, kais, pcen_g, pcen_o, pcen_e)

    W_all = _build_weight_array(W)
    KBB = _build_kbb(kais[0])
    Lm = _build_L()
    g = np.clip(pcen_g, 0.5, 0.999)
    o = np.clip(pcen_o, 0.0, 10.0)
    e = np.clip(pcen_e, 0.1, 1.0)
    par = np.stack(
        [o, e, g, g * o ** e, np.full(NF, 1e-6, np.float32)], axis=1
    ).astype(np.float32)
    idb = np.eye(128, dtype=np.float32).astype(BF16)
    idf = np.eye(128, dtype=np.float32)

    x2s = []
    for b in range(B):
        xpad = np.zeros(128 * X2C, np.float32)
        xpad[256 : 256 + T] = x[b, 0]
        x2s.append(np.ascontiguousarray(xpad.reshape(X2C, 128).T).astype(BF16))

    nc = _build_program()

    shared = {"W": W_all, "KBB": KBB, "IDB": idb, "IDF": idf, "PAR": par, "L": Lm}
    in_maps = [dict(shared, x2=x2s[b]) for b in range(B)]
    global LAST_NC, LAST_IN_MAPS
    LAST_NC = nc
    LAST_IN_MAPS = in_maps

    from concourse.bass_utils import run_bass_kernel_spmd

    trace = bool(int(os.environ.get("DFBL_TRACE", "0")))
    res = run_bass_kernel_spmd(
        nc, in_maps, list(range(N_CORES)), trace=trace
    )
    LAST_RESULT = res
    out = np.stack([res.results[b]["Y"] for b in range(B)], axis=0)
    return out.astype(np.float32)


# revision 9
# speedup vs baseline: 1.1573x; 1.1573x over previous
"""Trainium2 Bass kernel for the DFBL (Gabor filterbank + Kaiser pooling + PCEN) model.

Contract: kernel(**inputs) takes the FULL unsharded inputs
(x [8,1,160000], six [64] param vectors) and returns the FULL output
[8, 64, 1000] float32. Internally shards batch across 8 NeuronCores.

Algorithm (per core, one batch element):
  1. Gabor conv as matmuls via the residue decomposition t = 128u + s:
     out[n, 128u+s] = sum_d Wsd[q,n].T @ x2[q, u+d], where
     x2[q, c] = xpad[128c + q] is a time-minor layout of x loaded once,
     and Wsd are fp8e4m3 weight tiles (pow2 per-channel-pair scaled; the
     scale never needs undoing because PCEN is per-channel scale
     invariant). All weights stay SBUF-resident (8.7 MB).
  2. |.|^2 on the scalar engine into per-group s-major buffers (bf16).
  3. Block-sum pooling: the Kaiser window is approximated piecewise-
     constant over blocks of 8 samples (KB[j] = block mean). Because
     t = 128u + s, a block of 8 consecutive samples = 8 consecutive
     residues s at fixed u, so one grouped tensor_reduce per 8-residue
     group produces S8[m], m = 16u + g, written m-contiguous. This
     shrinks the transpose+pool-matmul PE work ~8x vs pooling at full
     rate.
  4. pooledT[tp, chan] accumulates in persistent PSUM banks via banded
     KB tiles (22 distinct [128,128] bf16 tiles).
  5. PCEN: EMA as a bf16 decay-matrix matmul, then the elementwise pow
     chain on ACT/DVE (same structure as the reference scan unrolled).
"""

import math
import os

import ml_dtypes
import numpy as np

SR = 16000
NF = 64
GK = 401
PK = 401
PSTRIDE = 160
PCEN_S = 0.025
FMIN = 30.0
FMAX = SR / 2.0 * 0.5
B, T = 8, 160000
TP = 1000
U = 1250  # T / 128
X2C = 1254  # x2 columns: u+d+2 for u<1250, d in [-2,2]
N_CORES = 8

BLK = 8           # pooling block size (samples)
NMB = T // BLK    # 20000 pooled blocks per core
KBN = (PK + BLK - 1) // BLK  # 51 block-kernel taps
STRB = PSTRIDE // BLK        # 20 blocks per pooled-output stride
OFFB = (PK // 2) // BLK      # 25 blocks of left context
SEGS = [(0, 256), (256, 512), (512, 768), (768, 1024), (1024, 1250)]

BF16 = ml_dtypes.bfloat16
E4M3 = ml_dtypes.float8_e4m3

# exposed for test.py
LAST_RESULT = None
LAST_NC = None
LAST_IN_MAPS = None


# ----------------------------------------------------------------- host math

def _softplus(x):
    return np.logaddexp(0.0, x)


def _host_filters(p_center, p_bw):
    """Wcat [128, 401] f32: rows 0-63 real, 64-127 imag, scaled by sqrt(0.5)."""
    half = (GK - 1) // 2
    t = np.arange(-half, half + 1, dtype=np.float64) / SR
    fc = np.clip(np.exp(p_center.astype(np.float64)), FMIN, FMAX - 10.0)
    bw_pos = _softplus(p_bw.astype(np.float64)) * 1000.0
    max_bw = 2.0 * np.minimum(fc - FMIN, FMAX - fc)
    bw = np.minimum(bw_pos, np.maximum(max_bw, 50.0))
    f_low = np.maximum(fc - 0.5 * bw, FMIN)
    f_high = np.minimum(fc + 0.5 * bw, FMAX)
    sigma = 0.5 / np.maximum(f_high - f_low, 20.0)
    env = np.exp(-0.5 * (t[None, :] / sigma[:, None]) ** 2)
    phase = 2.0 * np.pi * fc[:, None] * t[None, :]
    real_k = env * np.cos(phase)
    imag_k = env * np.sin(phase)
    W = np.concatenate([real_k, imag_k], axis=0) * np.sqrt(0.5)
    return W.astype(np.float32)


def _host_kaiser(beta):
    b = np.clip(beta.astype(np.float64), 1.0, 20.0)
    n = np.arange(PK, dtype=np.float64)
    arg = b[:, None] * np.sqrt(1.0 - (2.0 * n[None, :] / (PK - 1.0) - 1.0) ** 2)
    kais = np.i0(arg) / (np.i0(b)[:, None] + 1e-8)
    return kais.astype(np.float64)


def _valid_d(s):
    lo = int(math.ceil((s - 327) / 128))
    hi = (s + 200) // 128
    return list(range(lo, hi + 1))


def _woff():
    off, acc = [], 0
    for s in range(128):
        off.append(acc)
        acc += len(_valid_d(s))
    return off, acc


def _build_weight_array(W):
    """W_all [128, ntiles*128] fp8e4m3, tiles ordered (s asc, d asc).

    Per-channel pow2 scale (shared between the re/im of each filter, so
    re^2+im^2 stays consistently scaled); PCEN is per-channel scale
    invariant so the scale is never undone."""
    wmax = np.abs(W).max(axis=1)
    wmax = np.maximum(wmax[:NF], wmax[NF:])
    wmax = np.maximum(wmax, 1e-30)
    alpha = 2.0 ** np.floor(np.log2(176.0 / wmax))
    alpha = np.concatenate([alpha, alpha]).astype(np.float32)
    Ws = W * alpha[:, None]
    tiles = []
    for s in range(128):
        for d in _valid_d(s):
            tile = np.zeros((128, 128), np.float32)
            q = np.arange(128)
            k = 128 * d + q + 200 - s
            msk = (k >= 0) & (k < GK)
            tile[msk, :] = Ws[:, k[msk]].T
            tiles.append(tile)
    return np.concatenate(tiles, axis=1).astype(E4M3)


def _build_kbb(kr):
    """Banded block-kernel tiles [128, 22*128] bf16.

    KB[j] = mean of kr over block j (zero padded). Tile t (offset
    o = 128*(t-1)): T[m, tp] = KB[o + m - 20*tp + 25]."""
    kr_pad = np.zeros(KBN * BLK)
    kr_pad[:PK] = kr
    KB = kr_pad.reshape(KBN, BLK).mean(axis=1)
    m = np.arange(128)
    tp = np.arange(128)
    tiles = []
    for t in range(22):
        o = 128 * (t - 1)
        idx = o + m[:, None] - 20 * tp[None, :] + OFFB
        tile = np.where((idx >= 0) & (idx < KBN), KB[np.clip(idx, 0, KBN - 1)], 0.0)
        tiles.append(tile)
    return np.concatenate(tiles, axis=1).astype(BF16)


def _build_L():
    k_idx = np.arange(1024)
    tp_idx = np.arange(TP)
    Lm = np.where(
        (k_idx[:, None] <= tp_idx[None, :]) & (k_idx[:, None] < TP),
        PCEN_S * (1.0 - PCEN_S) ** np.clip(tp_idx[None, :] - k_idx[:, None], 0, None),
        0.0,
    )
    Ld = np.zeros((128, 8 * TP), np.float32)
    for blk in range(8):
        Ld[:, blk * TP:(blk + 1) * TP] = Lm[blk * 128:(blk + 1) * 128, :]
    return Ld.astype(BF16)


def _chunk_blocks():
    """Global pooling schedule: list of (C, mc, [(b, kbb_tile_idx), ...])."""
    out = []
    for C in range(157):
        m0 = 128 * C
        mc = min(128, NMB - m0)
        tp_lo = max(0, -(-(m0 - OFFB) // STRB))
        tp_hi = min(TP - 1, (m0 + mc - 1 + OFFB) // STRB)
        pairs = []
        if tp_lo <= tp_hi:
            for b in range(tp_lo // 128, tp_hi // 128 + 1):
                t = C - 20 * b + 1
                pairs.append((b, t))
        out.append((C, mc, pairs))
    return out


# ------------------------------------------------------------- device kernel

def _build_program():
    import concourse.bacc as bacc
    import concourse.bass as bass
    import concourse.mybir as mybir
    import concourse.tile as tile
    from concourse._compat import axon_active

    f32 = mybir.dt.float32
    bf16 = mybir.dt.bfloat16
    fp8 = mybir.dt.float8e4
    AF = mybir.ActivationFunctionType
    ALU = mybir.AluOpType

    woff, n_wtiles = _woff()
    sched = _chunk_blocks()

    # first/last pooling contribution per PSUM BANK (start zeroes the whole
    # bank, so flags must be bank-granular like the baseline)
    blk_first, blk_last = {}, {}
    for C, mc, pairs in sched:
        for b, t in pairs:
            bank = b // 4
            if bank not in blk_first:
                blk_first[bank] = (C, b)
            blk_last[bank] = (C, b)

    nc = bacc.Bacc(
        "TRN2",
        target_bir_lowering=False,
        debug=not axon_active(),
        num_devices=N_CORES,
    )

    x2_d = nc.dram_tensor("x2", [128, X2C], bf16, kind="ExternalInput").ap()
    w_d = nc.dram_tensor("W", [128, n_wtiles * 128], fp8, kind="ExternalInput").ap()
    kbb_d = nc.dram_tensor("KBB", [128, 22 * 128], bf16, kind="ExternalInput").ap()
    idb_d = nc.dram_tensor("IDB", [128, 128], bf16, kind="ExternalInput").ap()
    idf_d = nc.dram_tensor("IDF", [128, 128], f32, kind="ExternalInput").ap()
    par_d = nc.dram_tensor("PAR", [64, 5], f32, kind="ExternalInput").ap()
    l_d = nc.dram_tensor("L", [128, 8 * TP], bf16, kind="ExternalInput").ap()
    y_d = nc.dram_tensor("Y", [64, TP], f32, kind="ExternalOutput").ap()
    dbg = bool(int(os.environ.get("DFBL_DEBUG", "0")))
    if dbg:
        sq0_d = nc.dram_tensor("DSQ0", [128, 2048], bf16, kind="ExternalOutput").ap()
        sct_d = nc.dram_tensor("DSCT", [128, 512], bf16, kind="ExternalOutput").ap()
        sg0_d = nc.dram_tensor("DSG0", [128, 4096], bf16, kind="ExternalOutput").ap()
        ps_d = nc.dram_tensor("DPS", [128, 512], f32, kind="ExternalOutput").ap()
        ema_d = nc.dram_tensor("DEMA", [64, TP], f32, kind="ExternalOutput").ap()

    with tile.TileContext(nc) as tc:
        with (
            tc.tile_pool(name="const", bufs=1) as const_pool,
            tc.tile_pool(name="sqb", bufs=2) as sq_pool,
            tc.tile_pool(name="sg", bufs=2) as sg_pool,
            tc.tile_pool(name="sct", bufs=3) as sct_pool,
            tc.tile_pool(name="misc", bufs=1) as misc_pool,
            tc.tile_pool(name="psA", bufs=4, space="PSUM") as psA,
            tc.tile_pool(name="psB", bufs=2, space="PSUM") as psB,
            tc.tile_pool(name="psC", bufs=1, space="PSUM") as psC,
        ):
            x2_sb = const_pool.tile([128, X2C], bf16, tag="x2")
            w_sb = const_pool.tile([128, n_wtiles * 128], fp8, tag="w")
            # W chunk 0 + the x2 head first so conv group 0 starts ASAP
            wg_bounds = [woff[g] if g < 128 else n_wtiles for g in range(0, 129, 8)]
            nc.sync.dma_start(
                w_sb[:, 0 : wg_bounds[1] * 128], w_d[:, 0 : wg_bounds[1] * 128]
            )
            nc.sync.dma_start(x2_sb[:, 0:384], x2_d[:, 0:384])
            for i in range(1, 16):
                lo, hi = wg_bounds[i] * 128, wg_bounds[i + 1] * 128
                nc.sync.dma_start(w_sb[:, lo:hi], w_d[:, lo:hi])
            nc.sync.dma_start(x2_sb[:, 384:X2C], x2_d[:, 384:X2C])
            kbb_sb = const_pool.tile([128, 22 * 128], bf16, tag="kbb")
            nc.sync.dma_start(kbb_sb[:], kbb_d[:])
            idb_sb = const_pool.tile([128, 128], bf16, tag="idb")
            nc.sync.dma_start(idb_sb[:], idb_d[:])
            idf_sb = const_pool.tile([128, 128], f32, tag="idf")
            nc.sync.dma_start(idf_sb[:], idf_d[:])
            par_sb = const_pool.tile([64, 5], f32, tag="par")
            nc.sync.dma_start(par_sb[:], par_d[:])
            l_sb = const_pool.tile([128, 8 * TP], bf16, tag="L")
            nc.sync.dma_start(l_sb[:], l_d[:])

            pooled_ps = [
                psC.tile([128, 512], f32, tag=f"pool{i}", name=f"pool{i}")
                for i in range(2)
            ]

            gchunk = 0  # global chunk counter
            for (u0, u1) in SEGS:
                useg = u1 - u0
                sg = sg_pool.tile([128, 16 * 256], bf16, tag="sg", name="sg")
                sgv = sg[:].rearrange("p (u g) -> p u g", g=16)

                for g in range(16):  # 8-residue groups
                    sqb = sq_pool.tile([128, 8 * 256], bf16, tag="sqb", name="sqb")
                    for q in range(4):  # 2 s per PSUM bank
                        cps = psA.tile([128, 512], f32, tag="conv", name="cps")
                        for j in range(2):
                            s = 8 * g + 2 * q + j
                            ds = _valid_d(s)
                            for di, d in enumerate(ds):
                                nc.tensor.matmul(
                                    cps[:, j * useg : (j + 1) * useg],
                                    lhsT=w_sb[
                                        :,
                                        (woff[s] + di) * 128 : (woff[s] + di + 1) * 128,
                                    ],
                                    rhs=x2_sb[:, u0 + d + 2 : u0 + d + 2 + useg],
                                    start=(di == 0),
                                    stop=(di == len(ds) - 1),
                                    skip_group_check=True,
                                )
                        nc.scalar.activation(
                            sqb[:, 2 * q * useg : (2 * q + 2) * useg],
                            cps[:, 0 : 2 * useg],
                            AF.Square,
                        )
                    if dbg and u0 == 0 and g == 0:
                        nc.sync.dma_start(sq0_d[:], sqb[:, 0:2048])
                    sqv = sqb[:, 0 : 8 * useg].rearrange("p (s u) -> p u s", s=8)
                    uh = useg // 2
                    with nc.allow_low_precision(reason="8-sample pooling blocksum"):
                        nc.vector.tensor_reduce(
                            sgv[:, 0:uh, g], sqv[:, 0:uh, :],
                            op=ALU.add, axis=mybir.AxisListType.X,
                        )
                        nc.vector.tensor_reduce(
                            sgv[:, uh:useg, g], sqv[:, uh:useg, :],
                            op=ALU.add, axis=mybir.AxisListType.X,
                        )

                if dbg and u0 == 0:
                    nc.sync.dma_start(sg0_d[:], sg[:])
                # pooling for this segment
                nchunk = (16 * useg) // 128
                mrem = 16 * useg - nchunk * 128
                for cb in range(0, nchunk, 4):
                    n4 = min(4, nchunk - cb)
                    tp_ps = psB.tile([128, 512], bf16, tag="tp", name="tpps")
                    for j in range(n4):
                        nc.tensor.transpose(
                            tp_ps[:, j * 128 : (j + 1) * 128],
                            sg[:, (cb + j) * 128 : (cb + j + 1) * 128],
                            idb_sb[:],
                        )
                    sct = sct_pool.tile([128, 512], bf16, tag="sct", name="sct")
                    nc.vector.tensor_copy(sct[:, 0 : n4 * 128], tp_ps[:, 0 : n4 * 128])
                    if dbg and u0 == 0 and cb == 0:
                        nc.sync.dma_start(sct_d[:], sct[:])
                    for j in range(n4):
                        C, mc, pairs = sched[gchunk + cb + j]
                        for b, t in pairs:
                            nc.tensor.matmul(
                                pooled_ps[b // 4][:, (b % 4) * 128 : (b % 4 + 1) * 128],
                                lhsT=kbb_sb[:, t * 128 : (t + 1) * 128],
                                rhs=sct[:, j * 128 : (j + 1) * 128],
                                start=(blk_first[b // 4] == (C, b)),
                                stop=(blk_last[b // 4] == (C, b)),
                                skip_group_check=True,
                            )
                if mrem:
                    # partial chunk (last segment): mrem m-columns
                    tp_ps = psB.tile([128, 512], bf16, tag="tp", name="tpps")
                    nc.tensor.transpose(
                        tp_ps[0:mrem, 0:128],
                        sg[:, nchunk * 128 : nchunk * 128 + mrem],
                        idb_sb[:],
                    )
                    sct = sct_pool.tile([128, 512], bf16, tag="sct", name="sct")
                    nc.vector.tensor_copy(sct[0:mrem, 0:128], tp_ps[0:mrem, 0:128])
                    C, mc, pairs = sched[gchunk + nchunk]
                    assert mc == mrem
                    for b, t in pairs:
                        nc.tensor.matmul(
                            pooled_ps[b // 4][:, (b % 4) * 128 : (b % 4 + 1) * 128],
                            lhsT=kbb_sb[0:mrem, t * 128 : (t + 1) * 128],
                            rhs=sct[0:mrem, 0:128],
                            start=(blk_first[b // 4] == (C, b)),
                            stop=(blk_last[b // 4] == (C, b)),
                            skip_group_check=True,
                        )
                    gchunk += 1
                gchunk += nchunk

            # ---- PCEN tail ----
            pc = []
            for i in range(2):
                t = misc_pool.tile([128, 512], f32, tag=f"pc{i}")
                nc.vector.tensor_copy(t[:], pooled_ps[i][:])
                pc.append(t)
            poolsumT = misc_pool.tile([128, 512], f32, tag="pst")
            for blk in range(8):
                src = pc[blk // 4]
                col = (blk % 4) * 128
                nc.vector.tensor_add(
                    poolsumT[:, blk * 64 : (blk + 1) * 64],
                    src[:, col : col + 64],
                    src[:, col + 64 : col + 128],
                )
            if dbg:
                nc.sync.dma_start(ps_d[:], poolsumT[:])
            poolsumTb = misc_pool.tile([128, 512], bf16, tag="pstb")
            nc.gpsimd.tensor_copy(poolsumTb[:], poolsumT[:])

            pnm_ps = [
                psB.tile([64, 512], f32, tag="tp", name=f"pnm{_i}") for _i in range(2)
            ]
            for blk in range(8):
                nc.tensor.transpose(
                    pnm_ps[blk // 4][:, (blk % 4) * 128 : (blk % 4 + 1) * 128],
                    poolsumT[:, blk * 64 : (blk + 1) * 64],
                    idf_sb[:],
                )

            ema_ps = [
                psA.tile([64, 500], f32, tag="conv", name=f"ema{_i}") for _i in range(2)
            ]
            for blk in range(8):
                for half in range(2):
                    nc.tensor.matmul(
                        ema_ps[half][:],
                        lhsT=poolsumTb[:, blk * 64 : (blk + 1) * 64],
                        rhs=l_sb[:, blk * TP + half * 500 : blk * TP + (half + 1) * 500],
                        start=(blk == 0),
                        stop=(blk == 7),
                    )

            if dbg:
                emad = misc_pool.tile([64, TP], f32, tag="emad")
                nc.scalar.copy(emad[:, 0:500], ema_ps[0][:])
                nc.scalar.copy(emad[:, 500:TP], ema_ps[1][:])
                nc.sync.dma_start(ema_d[:], emad[:])
            t0 = misc_pool.tile([64, TP], f32, tag="t0")
            rec = misc_pool.tile([64, TP], f32, tag="rec")
            pnm = misc_pool.tile([64, TP], f32, tag="pnm")
            nc.scalar.copy(pnm[:, 0:512], pnm_ps[0][:])
            nc.scalar.copy(pnm[:, 512:TP], pnm_ps[1][:, 0:488])
            t2 = misc_pool.tile([64, TP], f32, tag="t2")
            t3 = misc_pool.tile([64, TP], f32, tag="t3")
            t4 = misc_pool.tile([64, TP], f32, tag="t4")
            y_sb = misc_pool.tile([64, TP], f32, tag="y")
            for qtr in range(4):
                hs = slice(qtr * 250, (qtr + 1) * 250)
                nc.scalar.activation(
                    t0[:, hs],
                    ema_ps[qtr // 2][:, (qtr % 2) * 250 : (qtr % 2 + 1) * 250],
                    AF.Identity,
                    bias=par_sb[:, 4:5],
                )
                nc.vector.reciprocal(rec[:, hs], t0[:, hs])
                nc.vector.tensor_mul(t2[:, hs], pnm[:, hs], rec[:, hs])
                nc.scalar.activation(
                    t3[:, hs], t2[:, hs], AF.Ln, bias=par_sb[:, 0:1], scale=1.0
                )
                nc.scalar.activation(
                    t4[:, hs], t3[:, hs], AF.Exp, bias=0.0, scale=par_sb[:, 1:2]
                )
                nc.vector.tensor_scalar(
                    y_sb[:, hs], t4[:, hs], par_sb[:, 2:3], par_sb[:, 3:4],
                    ALU.mult, ALU.subtract,
                )
                nc.sync.dma_start(y_d[:, hs], y_sb[:, hs])

    nc.compile()
    return nc


def _numpy_fallback(x, W, kais, pcen_g, pcen_o, pcen_e):
    """Correct-but-slow host path for non-uniform beta (never hit with the
    harness inputs, which use a uniform beta)."""
    out = np.zeros((B, NF, TP), np.float32)
    Wr, Wi = W[:NF] / np.sqrt(0.5), W[NF:] / np.sqrt(0.5)
    for b in range(B):
        xp = np.zeros(T + 2 * 200, np.float32)
        xp[200 : 200 + T] = x[b, 0]
        win = np.lib.stride_tricks.sliding_window_view(xp, GK)  # [T, GK]
        real = win @ Wr.T
        imag = win @ Wi.T
        scal = 0.5 * (real ** 2 + imag ** 2)  # [T, NF]
        sp = np.zeros((T + 2 * 200, NF), np.float32)
        sp[200 : 200 + T] = scal
        pooled = np.zeros((TP, NF), np.float32)
        for tp in range(TP):
            seg = sp[tp * PSTRIDE : tp * PSTRIDE + PK]
            pooled[tp] = np.einsum("kn,nk->n", seg, kais)
        g = np.clip(pcen_g, 0.5, 0.999)
        o = np.clip(pcen_o, 0.0, 10.0)
        e = np.clip(pcen_e, 0.1, 1.0)
        ema = np.zeros(NF, np.float32)
        for tp in range(TP):
            ema = (1.0 - PCEN_S) * ema + PCEN_S * pooled[tp]
            out[b, :, tp] = ((pooled[tp] / (ema + 1e-6) + o) ** e - o ** e) * g
    return out


def kernel(x, p_center, p_bw, beta, pcen_g, pcen_o, pcen_e):
    global LAST_RESULT
    x = np.asarray(x, np.float32)
    p_center = np.asarray(p_center, np.float32)
    p_bw = np.asarray(p_bw, np.float32)
    beta = np.asarray(beta, np.float32)
    pcen_g = np.asarray(pcen_g, np.float32)
    pcen_o = np.asarray(pcen_o, np.float32)
    pcen_e = np.asarray(pcen_e, np.float32)

    W = _host_filters(p_center, p_bw)
    kais = _host_kaiser(beta)
    if not np.all(kais == kais[0:1]):
        return _numpy_fallback(x, W, kais, pcen_g, pcen_o, pcen_e)

    W_all = _build_weight_array(W)
    KBB = _build_kbb(kais[0])
    Lm = _build_L()
    g = np.clip(pcen_g, 0.5, 0.999)
    o = np.clip(pcen_o, 0.0, 10.0)
    e = np.clip(pcen_e, 0.1, 1.0)
    par = np.stack(
        [o, e, g, g * o ** e, np.full(NF, 1e-6, np.float32)], axis=1
    ).astype(np.float32)
    idb = np.eye(128, dtype=np.float32).astype(BF16)
    idf = np.eye(128, dtype=np.float32)

    x2s = []
    for b in range(B):
        xpad = np.zeros(128 * X2C, np.float32)
        xpad[256 : 256 + T] = x[b, 0]
        x2s.append(np.ascontiguousarray(xpad.reshape(X2C, 128).T).astype(BF16))

    nc = _build_program()

    shared = {"W": W_all, "KBB": KBB, "IDB": idb, "IDF": idf, "PAR": par, "L": Lm}
    in_maps = [dict(shared, x2=x2s[b]) for b in range(B)]
    global LAST_NC, LAST_IN_MAPS
    LAST_NC = nc
    LAST_IN_MAPS = in_maps

    from concourse.bass_utils import run_bass_kernel_spmd

    trace = bool(int(os.environ.get("DFBL_TRACE", "0")))
    res = run_bass_kernel_spmd(
        nc, in_maps, list(range(N_CORES)), trace=trace
    )
    LAST_RESULT = res
    out = np.stack([res.results[b]["Y"] for b in range(B)], axis=0)
    return out.astype(np.float32)
